# revision 2
# baseline (speedup 1.0000x reference)
"""Bernstein flow density kernel for Trainium2 (8 NeuronCores, data-parallel).

Math (per sample x in R^5, per dim i):
  c = constrained(A_i)                     # [(4)^i, 15] monotone coeffs in (0,1)
  tf_k = sum_j cb_ij c[j,k]                # cb_i = multivariate Bernstein basis over x[:i]
  dcoef_k = tf_k - tf_{k-1}  (tf_{-1}=0, tf_15=1)
  db_k = 16*comb(15,k) x_i^k (1-x_i)^(15-k)
  f_i = sum_k dcoef_k db_k ;  density = prod_i f_i

Device mapping (v3 — all bases via exp-of-matmul, no DMA transposes,
ACT-instruction-count minimized):
  - ln x / ln(1-x) computed sample-major (ACT), split fp16 hi+lo (DVE).
  - per 128-sample subtile, the 20 ln values are transposed to
    basis-major with the PE transpose (matmul against identity).
  - ALL Bernstein bases (cb1..cb4 pure monomials; degree-15 derivative
    basis db) are exp(W @ lnT): integer-matrix matmuls over the hi/lo ln
    rows, one ACT Exp per PSUM bank (4 per 512-sample group).
  - dcoef via matmul with difference-folded weights (w13, w4a+w4b).
  - prod written in-place over the db rows (exp(0)=1 pad row 48 intact),
    one f matmul per group.
  - f values staged to SBUF; final prod_i via Ln/ones-matmul/Exp in one
    batched tail (2 activation-LUT switches per kernel).
"""

import math
import sys

import numpy as np

for _p in ("/opt/trn_rl_repo", "/root/.axon_site/_ro/trn_rl_repo"):
    if _p not in sys.path:
        sys.path.append(_p)

import concourse.bass as bass
import concourse.tile as tile
from concourse import bacc, mybir

F32 = mybir.dt.float32
F16 = mybir.dt.float16

DIM = 5
TF_DEG = 16
N_FULL = 262144
N_CORES = 8
N_CORE = N_FULL // N_CORES  # 32768
SC = 256.0  # scale folded into dcoef weights to keep fp16 away from subnormals
COMB3 = np.array([1.0, 3.0, 3.0, 1.0])
COMB15 = np.array([math.comb(15, k) for k in range(16)], dtype=np.float64)

# const-pack column offsets in cw16 [128, CW16_COLS] (fp16)
_IDENT = 0       # [128, 128]
_WBA = 128       # [20, 84]
_WBB = 212      # [20, 128]
_WBC = 340      # [20, 128]
_WBD = 468      # [20, 96]
_W13 = 564      # [84, 64]
_W4A = 628      # [128, 16]
_W4B = 644      # [128, 16]
_F12 = 660      # [96, 32]
CW16_COLS = 692
# cw32 [128, 6] (fp32): col0 expbias96, cols1:5 lnones, col5 fbias
CW32_COLS = 6


# ----------------------------------------------------------------- host consts
def _constrained(A):
    A = A.astype(np.float64)
    sp = np.log1p(np.exp(-np.abs(A))) + np.maximum(A, 0.0)  # softplus, stable
    cs = np.cumsum(sp, axis=1)
    return 2.0 * (1.0 / (1.0 + np.exp(-cs)) - 0.5)


def _dev_perm_scale(i):
    """Device row p (digit d of p = j_d, j_0 fastest) -> reference row
    (j_0 slowest: ref = sum_d j_d*4^(i-1-d)) + comb(3,.) product scale."""
    rows = 4**i
    ref_idx = np.zeros(rows, dtype=np.int64)
    scale = np.ones(rows)
    for p in range(rows):
        r = 0
        s = 1.0
        for d in range(i):
            jd = (p >> (2 * d)) & 3
            r += jd * 4 ** (i - 1 - d)
            s *= COMB3[jd]
        ref_idx[p] = r
        scale[p] = s
    return ref_idx, scale


def _dcoef_weights(C, combscale):
    """C: [rows,15] device-row-ordered coeffs; [rows,16] W with the
    tf-difference folded in, scaled so sum_j monomial_j W[j,k] = SC*dcoef_k."""
    rows = C.shape[0]
    W = np.zeros((rows, 16))
    W[:, 0] = C[:, 0]
    W[:, 1:15] = C[:, 1:15] - C[:, 0:14]
    W[:, 15] = 1.0 - C[:, 14]
    return W * combscale[:, None] * SC


def _basis_Q(i):
    """[10, 4^i] ln-coefficients for the pure-monomial cb_i device rows:
    row q in 0:5 -> ln x_q coeff (j_q), q in 5:10 -> ln(1-x_{q-5}) (3-j_q)."""
    R = 4**i
    Q = np.zeros((10, R))
    for p in range(R):
        for d in range(i):
            jd = (p >> (2 * d)) & 3
            Q[d, p] = jd
            Q[5 + d, p] = 3 - jd
    return Q


def _db_rowmap(r):
    """db row layout (96 rows): 0:48 d=1..3 k=0:16; 48:64 zero (exp->1);
    64:80 d=4; 80:96 d=0."""
    if r < 48:
        return 1 + r // 16, r % 16
    if r < 64:
        return None
    if r < 80:
        return 4, r - 64
    return 0, r - 80


def build_consts(A_list):
    Cs = []
    for i in range(DIM):
        C = _constrained(A_list[i])  # [(4)^i, 15] in reference row order
        if i == 0:
            Cs.append((C, np.ones(1)))
        else:
            ref_idx, scale = _dev_perm_scale(i)
            Cs.append((C[ref_idx], scale))
    Wd = [_dcoef_weights(Cperm, scale) for (Cperm, scale) in Cs]  # [rows,16]

    # fp16 dynamic range fix: scale each (dim, k) weight column by a power of
    # two so its max lands near 1024, fold the inverse into the db exp bias.
    colshift = np.zeros((5, 16))
    for i in range(5):
        m = np.max(np.abs(Wd[i]), axis=0)
        e = np.round(np.log2(1024.0 / np.maximum(m, 1e-300)))
        e = np.clip(e, -10, 40)
        Wd[i] = Wd[i] * np.exp2(e)[None, :]
        colshift[i] = e * math.log(2.0)

    w13 = np.zeros((84, 64))
    w13[0:4, 0:16] = Wd[1]
    w13[4:20, 16:32] = Wd[2]
    w13[20:84, 32:48] = Wd[3]
    w4a = Wd[4][0:128]
    w4b = Wd[4][128:256]

    # basis ln-coefficient matrices (hi rows 0:10, lo rows 10:20 identical)
    QA = np.concatenate([_basis_Q(1), _basis_Q(2), _basis_Q(3)], axis=1)  # 84
    Q4 = _basis_Q(4)
    wbA = np.vstack([QA, QA])  # [20, 84]
    wbB = np.vstack([Q4[:, 0:128], Q4[:, 0:128]])
    wbC = np.vstack([Q4[:, 128:256], Q4[:, 128:256]])
    wbD = np.zeros((20, 96))  # db args; cols 48:64 stay 0 (exp -> ones pad)
    for r in range(96):
        mk = _db_rowmap(r)
        if mk is None:
            continue
        d, k = mk
        for base in (0, 10):
            wbD[base + d, r] = float(k)
            wbD[base + 5 + d, r] = float(15 - k)

    expb = np.zeros((96, 1))
    for r in range(96):
        mk = _db_rowmap(r)
        if mk is None:
            continue
        d, k = mk
        expb[r, 0] = math.log(16.0 * COMB15[k]) - colshift[d, k]

    # one f matmul over dbT rows after the in-place prod:
    # rows 0:48 prod d1..3, 48 = exp(0)=1 pad, 49:64 zero, 64:80 prod d4,
    # 80:96 db d0 (dim-0 dcoef weights folded here).
    f12w = np.zeros((96, 32))
    for i in (1, 2, 3):
        f12w[(i - 1) * 16 : i * 16, i] = 1.0
    f12w[64:80, 4] = 1.0
    f12w[80:96, 0] = Wd[0][0]
    f12w[48, 5:32] = 1.0  # psum pad rows stay ln-safe

    lnones = np.zeros((128, 4))
    for t in range(4):
        lnones[32 * t : 32 * t + 5, t] = 1.0
    fbias = np.full((4, 1), -DIM * math.log(SC))

    cw16 = np.zeros((128, CW16_COLS), dtype=np.float16)
    cw16[0:128, _IDENT : _IDENT + 128] = np.eye(128)
    cw16[0:20, _WBA : _WBA + 84] = wbA
    cw16[0:20, _WBB : _WBB + 128] = wbB
    cw16[0:20, _WBC : _WBC + 128] = wbC
    cw16[0:20, _WBD : _WBD + 96] = wbD
    cw16[0:84, _W13 : _W13 + 64] = w13
    cw16[0:128, _W4A : _W4A + 16] = w4a
    cw16[0:128, _W4B : _W4B + 16] = w4b
    cw16[0:96, _F12 : _F12 + 32] = f12w

    cw32 = np.zeros((128, CW32_COLS), dtype=np.float32)
    cw32[0:96, 0] = expb[:, 0]
    cw32[0:128, 1:5] = lnones
    cw32[0:4, 5] = fbias[:, 0]
    return {"cw16": cw16, "cw32": cw32}


# ---------------------------------------------------------------- device build
def build_nc(ncore):
    assert ncore % 2048 == 0
    nsub = ncore // 128  # 128-sample subtiles
    ngroup = nsub // 4  # 512-sample groups
    nsb = ngroup // 4  # 2048-sample superblocks
    xcols = nsub * 5

    nc = bacc.Bacc("TRN2", target_bir_lowering=False, debug=False, num_devices=N_CORES)
    xt = nc.declare_dram_parameter("xt", [128, xcols], F32, isOutput=False)
    cw16 = nc.declare_dram_parameter("cw16", [128, CW16_COLS], F16, isOutput=False)
    cw32 = nc.declare_dram_parameter("cw32", [128, CW32_COLS], F32, isOutput=False)
    dens = nc.declare_dram_parameter("dens", [ncore], F32, isOutput=True)

    Exp = mybir.ActivationFunctionType.Exp
    Ln = mybir.ActivationFunctionType.Ln

    with tile.TileContext(nc) as tc:
        with (
            tc.tile_pool(name="wc", bufs=1) as wc,
            tc.tile_pool(name="gr", bufs=2) as gr,
            tc.tile_pool(name="sbp", bufs=2) as sbp,
            tc.tile_pool(name="pT", bufs=1, space="PSUM") as pT,
            tc.tile_pool(name="pA", bufs=1, space="PSUM") as pA,
            tc.tile_pool(name="pB", bufs=1, space="PSUM") as pB,
            tc.tile_pool(name="pC", bufs=1, space="PSUM") as pC,
            tc.tile_pool(name="pD", bufs=1, space="PSUM") as pD,
            tc.tile_pool(name="pE", bufs=1, space="PSUM") as pE,
            tc.tile_pool(name="pF", bufs=1, space="PSUM") as pF,
            tc.tile_pool(name="pG", bufs=1, space="PSUM") as pG,
        ):
            cwsb = wc.tile([128, CW16_COLS], F16, tag="cwsb")
            cwf = wc.tile([128, CW32_COLS], F32, tag="cwf")
            xall = wc.tile([128, xcols], F32, tag="xall")
            for dst, src in ((cwsb, cw16), (cwf, cw32), (xall, xt)):
                nc.gpsimd.dma_start(out=dst[:], in_=src[:])

            ident = cwsb[:, _IDENT : _IDENT + 128]
            wbA = cwsb[0:20, _WBA : _WBA + 84]
            wbB = cwsb[0:20, _WBB : _WBB + 128]
            wbC = cwsb[0:20, _WBC : _WBC + 128]
            wbD = cwsb[0:20, _WBD : _WBD + 96]
            w13 = cwsb[0:84, _W13 : _W13 + 64]
            w4a = cwsb[:, _W4A : _W4A + 16]
            w4b = cwsb[:, _W4B : _W4B + 16]
            f12w = cwsb[0:96, _F12 : _F12 + 32]
            lnones = cwf[:, 1:5]
            fbias = cwf[0:4, 5:6]

            xa = xall[:].rearrange("p (n d) -> p n d", d=5)  # [128, nsub, 5]
            ln32 = wc.tile([128, nsub, 10], F32, tag="ln32")
            lnhl = wc.tile([128, nsub, 20], F16, tag="lnhl")
            flnall = wc.tile([128, nsb, 512], F32, tag="flnall")
            lnall = wc.tile([128, nsb, 512], F32, tag="lnall")
            nc.scalar.activation(out=ln32[:, :, 0:5], in_=xa, func=Ln)
            nc.scalar.activation(
                out=ln32[:, :, 5:10], in_=xa, func=Ln, scale=-1.0, bias=1.0
            )
            nc.vector.tensor_copy(out=lnhl[:, :, 0:10], in_=ln32[:])
            nc.vector.tensor_sub(
                out=lnhl[:, :, 10:20], in0=ln32[:], in1=lnhl[:, :, 0:10]
            )

            for g in range(ngroup):
                tp = g % 4
                psT = pT.tile([20, 512], F16, tag="psT")
                for t in range(4):
                    nc.tensor.transpose(
                        out=psT[:, t * 128 : (t + 1) * 128],
                        in_=lnhl[:, 4 * g + t, :],
                        identity=ident,
                    )
                lnT = gr.tile([20, 512], F16, tag="lnT")
                nc.vector.tensor_copy(out=lnT[:], in_=psT[:])

                psA = pA.tile([84, 512], F32, tag="psA")
                psB = pB.tile([128, 512], F32, tag="psB")
                psC = pC.tile([128, 512], F32, tag="psC")
                psD = pD.tile([96, 512], F32, tag="psD")
                nc.tensor.matmul(out=psA[:], lhsT=wbA, rhs=lnT[:], start=True, stop=True)
                nc.tensor.matmul(out=psB[:], lhsT=wbB, rhs=lnT[:], start=True, stop=True)
                nc.tensor.matmul(out=psC[:], lhsT=wbC, rhs=lnT[:], start=True, stop=True)
                nc.tensor.matmul(out=psD[:], lhsT=wbD, rhs=lnT[:], start=True, stop=True)

                cb123 = gr.tile([84, 512], F16, tag="cb123")
                cb4a = gr.tile([128, 512], F16, tag="cb4a")
                cb4b = gr.tile([128, 512], F16, tag="cb4b")
                dbT = gr.tile([96, 512], F16, tag="dbT")
                nc.scalar.activation(out=cb123[:], in_=psA[:], func=Exp)
                nc.scalar.activation(out=cb4a[:], in_=psB[:], func=Exp)
                nc.scalar.activation(out=cb4b[:], in_=psC[:], func=Exp)
                nc.scalar.activation(
                    out=dbT[:], in_=psD[:], func=Exp, bias=cwf[0:96, 0:1]
                )

                dtfp = pE.tile([80, 512], F32, tag="dtfp")
                nc.tensor.matmul(
                    out=dtfp[0:64, :], lhsT=w13, rhs=cb123[:], start=True, stop=True
                )
                nc.tensor.matmul(
                    out=dtfp[64:80, :], lhsT=w4a, rhs=cb4a[:], start=True, stop=False
                )
                nc.tensor.matmul(
                    out=dtfp[64:80, :], lhsT=w4b, rhs=cb4b[:], start=False, stop=True
                )
                # in-place prod over the db rows; row 48 (=1.0 pad) and the d0
                # rows 80:96 stay untouched (32-aligned DVE starts).
                nc.vector.tensor_mul(
                    out=dbT[0:48, :], in0=dtfp[0:48, :], in1=dbT[0:48, :]
                )
                nc.vector.tensor_mul(
                    out=dbT[64:80, :], in0=dtfp[64:80, :], in1=dbT[64:80, :]
                )

                if tp == 0:
                    fpsum = pF.tile([128, 512], F32, tag="fpsum")
                nc.tensor.matmul(
                    out=fpsum[32 * tp : 32 * tp + 32, :],
                    lhsT=f12w,
                    rhs=dbT[:],
                    start=True,
                    stop=True,
                    tile_position=(0, 32 * tp),
                )
                if tp == 3:
                    sb = g // 4
                    nc.vector.tensor_copy(out=flnall[:, sb, :], in_=fpsum[:])

            # batched tail: 2 LUT switches total
            nc.scalar.activation(out=lnall[:], in_=flnall[:], func=Ln)
            for sb in range(nsb):
                lnden = pG.tile([4, 512], F32, tag="lnden")
                nc.tensor.matmul(
                    out=lnden[:],
                    lhsT=lnones,
                    rhs=lnall[:, sb, :],
                    start=True,
                    stop=True,
                )
                dens_sb = sbp.tile([4, 512], F32, tag="dens_sb")
                nc.scalar.activation(out=dens_sb[:], in_=lnden[:], func=Exp, bias=fbias)
                base = sb * 2048
                nc.gpsimd.dma_start(
                    out=dens[base : base + 2048].rearrange("(t s) -> t s", t=4),
                    in_=dens_sb[:],
                )
    nc.finalize()
    return nc


# ------------------------------------------------------------ sharded runner
def _shard_map(f, mesh, in_specs, out_specs):
    import jax

    try:
        return jax.shard_map(
            f, mesh=mesh, in_specs=in_specs, out_specs=out_specs, check_vma=False
        )
    except Exception:
        from jax.experimental.shard_map import shard_map as _sm

        return _sm(
            f, mesh=mesh, in_specs=in_specs, out_specs=out_specs, check_rep=False
        )


_CACHE = {}


def _get_runner():
    """Build nc + a cached fast-dispatch sharded callable (compile once)."""
    if "runner" in _CACHE:
        return _CACHE["runner"]
    import jax
    from jax.sharding import Mesh, PartitionSpec

    from concourse import mybir as _mb
    from concourse.bass2jax import (
        _bass_exec_p,
        install_neuronx_cc_hook,
        partition_id_tensor,
    )

    install_neuronx_cc_hook()
    nc = build_nc(N_CORE)
    partition_name = nc.partition_id_tensor.name if nc.partition_id_tensor else None

    in_names, out_names, out_avals, in_shapes = [], [], [], []
    for alloc in nc.m.functions[0].allocations:
        if not isinstance(alloc, _mb.MemoryLocationSet):
            continue
        name = alloc.memorylocations[0].name
        if alloc.kind == "ExternalInput":
            if name != partition_name:
                in_names.append(name)
                in_shapes.append(
                    (tuple(alloc.tensor_shape), _mb.dt.np(alloc.dtype))
                )
        elif alloc.kind == "ExternalOutput":
            out_names.append(name)
            out_avals.append(
                jax.core.ShapedArray(
                    tuple(alloc.tensor_shape), _mb.dt.np(alloc.dtype)
                )
            )
    all_in_names = list(in_names) + list(out_names)
    if partition_name is not None:
        all_in_names.append(partition_name)

    def _body(*args):
        operands = list(args)
        if partition_name is not None:
            operands.append(partition_id_tensor())
        outs = _bass_exec_p.bind(
            *operands,
            out_avals=tuple(out_avals),
            in_names=tuple(all_in_names),
            out_names=tuple(out_names),
            lowering_input_output_aliases=(),
            sim_require_finite=True,
            sim_require_nnan=True,
            nc=nc,
        )
        return tuple(outs)

    devices = jax.devices()[:N_CORES]
    mesh = Mesh(np.asarray(devices), ("core",))
    shard = jax.NamedSharding(mesh, PartitionSpec("core"))
    n_in = len(in_names) + len(out_names)

    def build_jit():
        return jax.jit(
            _shard_map(
                _body,
                mesh,
                (PartitionSpec("core"),) * n_in,
                (PartitionSpec("core"),) * len(out_avals),
            ),
            keep_unused=True,
        )

    structs = [
        jax.ShapeDtypeStruct((N_CORES * s[0], *s[1:]), dt, sharding=shard)
        for (s, dt) in in_shapes
    ] + [
        jax.ShapeDtypeStruct(
            (N_CORES * a.shape[0], *a.shape[1:]), a.dtype, sharding=shard
        )
        for a in out_avals
    ]
    try:
        from concourse.bass2jax import fast_dispatch_compile

        call = fast_dispatch_compile(lambda: build_jit().lower(*structs).compile())
    except Exception:
        call = build_jit()

    zeros_dev = [
        jax.device_put(
            np.zeros((N_CORES * a.shape[0], *a.shape[1:]), a.dtype), shard
        )
        for a in out_avals
    ]
    _CACHE["runner"] = (call, in_names, out_names, out_avals, zeros_dev, shard)
    return _CACHE["runner"]


def pack_x(x_shard):
    """[n, 5] -> [128, n/128*5]; sample s = nb*128+p -> row p, cols nb*5+d."""
    n = x_shard.shape[0]
    return (
        np.ascontiguousarray(x_shard.reshape(n // 128, 128, 5).transpose(1, 0, 2))
        .reshape(128, n // 128 * 5)
        .astype(np.float32)
    )


def make_in_maps(x, A_list):
    consts = build_consts([np.asarray(a, dtype=np.float64) for a in A_list])
    in_maps = []
    for c in range(N_CORES):
        m = {"xt": pack_x(x[c * N_CORE : (c + 1) * N_CORE])}
        m.update(consts)
        in_maps.append(m)
    return in_maps


def device_args(x, A_list):
    """Concatenated + device_put sharded inputs (incl. zero output inits)."""
    import jax

    call, in_names, out_names, out_avals, zeros_dev, shard = _get_runner()
    in_maps = make_in_maps(x, A_list)
    concat_in = [
        jax.device_put(
            np.concatenate(
                [np.asarray(in_maps[c][k]) for c in range(N_CORES)], axis=0
            ),
            shard,
        )
        for k in in_names
    ]
    return list(concat_in) + list(zeros_dev)


def kernel(x, A0, A1, A2, A3, A4):
    x = np.asarray(x, dtype=np.float32)
    call, in_names, out_names, out_avals, zeros_dev, shard = _get_runner()
    args = device_args(x, (A0, A1, A2, A3, A4))
    outs = call(*args)
    dens = np.asarray(outs[0]).reshape(-1)
    return dens.astype(np.float32)
